# revision 1
# baseline (speedup 1.0000x reference)
"""MergedQKVParallelLinearWithLora on 8 TRN2 NeuronCores.

Strategy: token-parallel (data-parallel) across the 8 cores — each core
computes 4096 tokens of the full (T=32768, O=3072) output. Per core:

  out^T = W^T-accumulated f32r matmuls (K=2048 contraction over D)
        + lora expand (K=128, bf16)  + lora bias (K=8, bf16)
        + per-channel bias (DVE tensor_scalar_add at PSUM eviction)

The lora shrink s = x @ A^T is computed on-device for all 3 slices and all
8 adapters at once (A stacked to (384, D)), masked per-token by the
adapter one-hot (so tokens with idx==-1 or a different adapter contribute
zero), stored bf16, and consumed as the moving operand of the expand matmul.

Layouts are prepared host-side: x, W, A fed transposed so the contraction
dim D lands on SBUF partitions; output comes back as out^T per core and is
transposed/concatenated on the host.

All SBUF operands are k-tile-granular (128-partition tiles) so DMA/compute
dependencies stay fine-grained: the first matmul issues ~2us in, W tiles for
the next output pass prefetch while the current pass drains, and the x pool
double-buffers token tiles across all four passes.
"""

import numpy as np
import ml_dtypes

import concourse.mybir as mybir
import concourse.tile as tile
from concourse import bacc
from concourse.bass_utils import run_bass_kernel_spmd

T, D, QS, KVS, L, R = 32768, 2048, 2048, 512, 8, 16
O = QS + 2 * KVS          # 3072
NCORES = 8
TC = T // NCORES          # 4096 tokens per core
NT = 512                  # tokens per tile (matmul moving dim)
NKT = D // 128            # 16 contraction k-tiles
NBLK = O // 128           # 24 output-channel blocks
WBLK = 8                  # blocks per W pass (3 passes)

F32 = mybir.dt.float32
F32R = mybir.dt.float32r
BF16 = mybir.dt.bfloat16
BF16NP = ml_dtypes.bfloat16


def build_program(tc_tokens=TC):
    ntt = tc_tokens // NT
    nc = bacc.Bacc(None, target_bir_lowering=False, debug=False)

    xT = nc.dram_tensor("xT", [D, tc_tokens], F32R, kind="ExternalInput")
    wT = nc.dram_tensor("wT", [D, O], F32R, kind="ExternalInput")
    aT = nc.dram_tensor("aT", [D, 3 * 128], F32R, kind="ExternalInput")
    bcomb = nc.dram_tensor("bcomb", [128, O], BF16, kind="ExternalInput")
    biasL = nc.dram_tensor("biasL", [L, O], BF16, kind="ExternalInput")
    bias_arr = nc.dram_tensor("bias_arr", [128, NBLK], F32, kind="ExternalInput")
    maskT = nc.dram_tensor("maskT", [128, tc_tokens], BF16, kind="ExternalInput")
    ohT = nc.dram_tensor("ohT", [L, tc_tokens], BF16, kind="ExternalInput")
    outT = nc.dram_tensor("outT", [O, tc_tokens], F32, kind="ExternalOutput")

    with tile.TileContext(nc) as tc:
        with tc.tile_pool(name="const", bufs=1) as const, \
             tc.tile_pool(name="xp", bufs=8) as xp, \
             tc.tile_pool(name="wp", bufs=4) as wp, \
             tc.tile_pool(name="psm", bufs=8, space="PSUM") as psm, \
             tc.tile_pool(name="op", bufs=4) as op:
            st_all = [const.tile([128, tc_tokens], BF16, tag=f"st{s}", name=f"st{s}")
                      for s in range(3)]
            bc_t = const.tile([128, O], BF16, tag="bc")
            bl_t = const.tile([L, O], BF16, tag="bl")
            oh_t = const.tile([L, tc_tokens], BF16, tag="oh")
            ba_t = const.tile([128, NBLK], F32, tag="ba")

            # quad-batched loads: one DMA covers 4 contraction k-tiles, so the
            # sync sequencer issues 4 descriptors per token tile instead of 16
            def load_x(tt):
                ts = []
                for q in range(NKT // 4):
                    t = xp.tile([128, 4, NT], F32R, tag="x", name=f"x_t{tt}_q{q}")
                    nc.sync.dma_start(
                        out=t[:],
                        in_=xT[:, tt * NT:(tt + 1) * NT].rearrange(
                            "(i p) n -> p i n", p=128)[:, q * 4:(q + 1) * 4, :])
                    ts.append(t)
                return lambda i: ts[i // 4][:, i % 4, :]

            def load_w(p):
                ts = []
                for q in range(NKT // 4):
                    t = wp.tile([128, 4, WBLK * 128], F32R, tag="w",
                                name=f"w_p{p}_q{q}")
                    nc.sync.dma_start(
                        out=t[:],
                        in_=wT[:, p * WBLK * 128:(p + 1) * WBLK * 128].rearrange(
                            "(i p) n -> p i n", p=128)[:, q * 4:(q + 1) * 4, :])
                    ts.append(t)
                return lambda i: ts[i // 4][:, i % 4, :]

            # ---- shrink pass: s~ = mask * (x @ A^T), all tokens, bf16 ----
            # (the main passes' first W tiles prefetch during this pass: the
            # wp pool is open and its slots are free)
            with tc.tile_pool(name="shr", bufs=1) as shr, \
                 tc.tile_pool(name="mkp", bufs=2) as mkp:
                # critical path first: x(tt0) + A feed the very first matmuls
                x_first = load_x(0)
                a_qs = []
                for q in range(NKT // 4):
                    t = shr.tile([128, 4, 384], F32R, tag=f"a{q}", name=f"a_q{q}")
                    nc.sync.dma_start(
                        out=t[:],
                        in_=aT.rearrange("(i p) n -> p i n", p=128)[:, q * 4:(q + 1) * 4, :])
                    a_qs.append(t)
                a_ts = lambda i: a_qs[i // 4][:, i % 4, :]
                nc.gpsimd.dma_start(out=bc_t[:], in_=bcomb[:])
                nc.gpsimd.dma_start(out=bl_t[:], in_=biasL[:])
                nc.gpsimd.dma_start(out=oh_t[:], in_=ohT[:])
                nc.gpsimd.dma_start(out=ba_t[:], in_=bias_arr[:])
                w_next = load_w(0)
                for tt in range(ntt):
                    x_ts = x_first if tt == 0 else load_x(tt)
                    mk_t = mkp.tile([128, NT], BF16, tag="mk")
                    nc.gpsimd.dma_start(
                        out=mk_t[:], in_=maskT[:, tt * NT:(tt + 1) * NT])
                    for s in range(3):
                        ps = psm.tile([128, NT], F32, tag="ps")
                        for i in range(NKT):
                            nc.tensor.matmul(
                                ps[:],
                                a_ts(i)[:, s * 128:(s + 1) * 128],
                                x_ts(i),
                                start=(i == 0), stop=(i == NKT - 1),
                            )
                        nc.vector.tensor_mul(
                            st_all[s][:, tt * NT:(tt + 1) * NT], ps[:], mk_t[:])

            # ---- main passes: 3 x (8 channel-blocks over all tokens) ----
            for p in range(NBLK // WBLK):
                w_ts = w_next
                w_next = load_w(p + 1) if p + 1 < NBLK // WBLK else None
                for tt in range(ntt):
                    x_ts = load_x(tt)
                    # bf16 lora matmuls for all 8 blocks first (one PSUM bank
                    # each: datatype switches only twice per token tile), then
                    # per-block f32r runs with trailing evictions so each
                    # bank frees long before the next tile's bf16 matmul
                    # needs it
                    pss = []
                    for blk in range(WBLK):
                        j = p * WBLK + blk
                        s = 0 if j < QS // 128 else (1 if j < (QS + KVS) // 128 else 2)
                        ps = psm.tile([128, NT], F32, tag="ps", name=f"ps{j}_{tt}")
                        pss.append(ps)
                        nc.tensor.matmul(
                            ps[:],
                            bl_t[:, j * 128:(j + 1) * 128],
                            oh_t[:, tt * NT:(tt + 1) * NT],
                            start=True, stop=False, skip_group_check=True,
                        )
                        nc.tensor.matmul(
                            ps[:],
                            bc_t[:, j * 128:(j + 1) * 128],
                            st_all[s][:, tt * NT:(tt + 1) * NT],
                            start=False, stop=False, skip_group_check=True,
                        )
                    for blk in range(WBLK):
                        j = p * WBLK + blk
                        for i in range(NKT):
                            nc.tensor.matmul(
                                pss[blk][:],
                                w_ts(i)[:, blk * 128:(blk + 1) * 128],
                                x_ts(i),
                                start=False, stop=(i == NKT - 1),
                                skip_group_check=True,
                            )
                        o_t = op.tile([128, NT], F32, tag="o")
                        nc.vector.tensor_scalar_add(o_t[:], pss[blk][:], ba_t[:, j:j + 1])
                        nc.gpsimd.dma_start(
                            out=outT[j * 128:(j + 1) * 128, tt * NT:(tt + 1) * NT],
                            in_=o_t[:],
                        )
    nc.compile()
    return nc


_nc_cache = {}


def _get_program(tc_tokens=TC):
    if tc_tokens not in _nc_cache:
        _nc_cache[tc_tokens] = build_program(tc_tokens)
    return _nc_cache[tc_tokens]


def make_in_maps(x, W_qkv, bias_qkv, lora_a_q, lora_a_k, lora_a_v,
                 lora_b_q, lora_b_k, lora_b_v,
                 lora_bias_q, lora_bias_k, lora_bias_v,
                 token_lora_indices, ncores=NCORES):
    x = np.asarray(x, np.float32)
    idx = np.asarray(token_lora_indices).astype(np.int64)
    tc_tokens = x.shape[0] // ncores

    wT = np.ascontiguousarray(np.asarray(W_qkv, np.float32).T)
    a_stack = np.concatenate([
        np.asarray(lora_a_q, np.float32).reshape(L * R, D),
        np.asarray(lora_a_k, np.float32).reshape(L * R, D),
        np.asarray(lora_a_v, np.float32).reshape(L * R, D)], axis=0)
    aT = np.ascontiguousarray(a_stack.T)
    bcomb = np.concatenate([
        np.asarray(lora_b_q, np.float32).transpose(0, 2, 1).reshape(L * R, QS),
        np.asarray(lora_b_k, np.float32).transpose(0, 2, 1).reshape(L * R, KVS),
        np.asarray(lora_b_v, np.float32).transpose(0, 2, 1).reshape(L * R, KVS)],
        axis=1).astype(BF16NP)
    biasL = np.concatenate([
        np.asarray(lora_bias_q, np.float32),
        np.asarray(lora_bias_k, np.float32),
        np.asarray(lora_bias_v, np.float32)], axis=1).astype(BF16NP)
    bias_arr = np.ascontiguousarray(
        np.asarray(bias_qkv, np.float32).reshape(NBLK, 128).T)
    lane = np.arange(128) // R

    in_maps = []
    for c in range(ncores):
        sl = slice(c * tc_tokens, (c + 1) * tc_tokens)
        idx_c = idx[sl]
        in_maps.append({
            "xT": np.ascontiguousarray(x[sl].T),
            "wT": wT,
            "aT": aT,
            "bcomb": bcomb,
            "biasL": biasL,
            "bias_arr": bias_arr,
            "maskT": (idx_c[None, :] == lane[:, None]).astype(BF16NP),
            "ohT": (idx_c[None, :] == np.arange(L)[:, None]).astype(BF16NP),
        })
    return in_maps, tc_tokens


def kernel(x, W_qkv, bias_qkv, lora_a_q, lora_a_k, lora_a_v,
           lora_b_q, lora_b_k, lora_b_v,
           lora_bias_q, lora_bias_k, lora_bias_v,
           token_lora_indices):
    in_maps, tc_tokens = make_in_maps(
        x, W_qkv, bias_qkv, lora_a_q, lora_a_k, lora_a_v,
        lora_b_q, lora_b_k, lora_b_v,
        lora_bias_q, lora_bias_k, lora_bias_v, token_lora_indices)
    nc = _get_program(tc_tokens)
    res = run_bass_kernel_spmd(nc, in_maps, list(range(NCORES)))
    out = np.empty((T, O), np.float32)
    for c in range(NCORES):
        out[c * tc_tokens:(c + 1) * tc_tokens] = res.results[c]["outT"].T
    return out



# revision 2
# speedup vs baseline: 1.2149x; 1.2149x over previous
"""MergedQKVParallelLinearWithLora on 8 TRN2 NeuronCores.

Strategy: token-parallel (data-parallel) across the 8 cores — each core
computes 4096 tokens of the full (T=32768, O=3072) output.

v2 layout (vs the f32r baseline at ~1005us):
  - main GEMM in bf16 (same 1 cyc/row PE rate as f32r, but LDWEIGHTS is
    4x shorter so it fully shadow-loads under the previous matmul)
  - W fully SBUF-resident (12.5MB bf16), single pass over x: per-core
    HBM traffic drops ~164MB -> ~66MB, no pass-boundary DMA stalls
  - lora shrink in fp8e4 DoubleRow: 2 k-tiles per matmul instruction
    (half the instructions of the f32r shrink)
  - lora expand + lora bias fused into ONE fp8e4 DoubleRow matmul per
    output block: plane 0 = bcomb x st, plane 1 = biasL x one-hot
    (zero-padded to 128 partitions)
  - fp8 scaling keeps everything in e4m3 normal range: A,B,biasL are
    scaled x8 host-side; the shrink mask is 1/64 (undoes A's x8 and
    pre-divides by 8), the one-hot is 1/8
  - output stored bf16 (halves writeback; host converts back to f32)

Per-core PE work: 192 shrink DR + 192 fused-expand DR + 3072 bf16 main
matmuls of 512 moving rows each ~= 755us at 2.4GHz / 1 cyc/row.
"""

import numpy as np
import ml_dtypes

import concourse.mybir as mybir
import concourse.tile as tile
from concourse import bacc
from concourse.bass_utils import run_bass_kernel_spmd

T, D, QS, KVS, L, R = 32768, 2048, 2048, 512, 8, 16
O = QS + 2 * KVS          # 3072
NCORES = 8
TC = T // NCORES          # 4096 tokens per core
NT = 512                  # tokens per tile (matmul moving dim)
NKT = D // 128            # 16 contraction k-tiles
NQP = NKT // 2            # 8 DoubleRow k-tile pairs
NBLK = O // 128           # 24 output-channel blocks

F32 = mybir.dt.float32
BF16 = mybir.dt.bfloat16
FP8 = mybir.dt.float8e4
DR = mybir.MatmulPerfMode.DoubleRow
BF16NP = ml_dtypes.bfloat16
FP8NP = ml_dtypes.float8_e4m3


def build_program(tc_tokens=TC):
    ntt = tc_tokens // NT
    nc = bacc.Bacc(None, target_bir_lowering=False, debug=False)

    xb = nc.dram_tensor("xb", [128, NKT, tc_tokens], BF16, kind="ExternalInput")
    x8 = nc.dram_tensor("x8", [128, NQP, 2, tc_tokens], FP8, kind="ExternalInput")
    wres = nc.dram_tensor("wres", [128, NKT, O], BF16, kind="ExternalInput")
    a8 = nc.dram_tensor("a8", [128, NQP, 2, 384], FP8, kind="ExternalInput")
    wc8 = nc.dram_tensor("wc8", [128, NBLK, 2, 128], FP8, kind="ExternalInput")
    oh8 = nc.dram_tensor("oh8", [128, tc_tokens], FP8, kind="ExternalInput")
    mask8 = nc.dram_tensor("mask8", [128, tc_tokens], FP8, kind="ExternalInput")
    bias_arr = nc.dram_tensor("bias_arr", [128, NBLK], F32, kind="ExternalInput")
    outT = nc.dram_tensor("outT", [O, tc_tokens], BF16, kind="ExternalOutput")

    with tile.TileContext(nc) as tc:
        with tc.tile_pool(name="const", bufs=1) as const, \
             tc.tile_pool(name="xp", bufs=2) as xp, \
             tc.tile_pool(name="x8p", bufs=2) as x8p, \
             tc.tile_pool(name="mkp", bufs=2) as mkp, \
             tc.tile_pool(name="psm", bufs=8, space="PSUM") as psm, \
             tc.tile_pool(name="op", bufs=4) as op:
            # st_oh[s]: interleaved moving operand for the fused expand
            # matmul: [:, tt, 0, :] = masked shrink (s/8, fp8),
            # [:, tt, 1, :] = padded one-hot / 8
            st_oh = [const.tile([128, ntt, 2, NT], FP8, tag=f"st{s}",
                                name=f"st{s}") for s in range(3)]
            a8_t = const.tile([128, NQP, 2, 384], FP8, tag="a8")
            wc8_t = const.tile([128, NBLK, 2, 128], FP8, tag="wc8")
            ba_t = const.tile([128, NBLK], F32, tag="ba")
            w_t = const.tile([128, NKT, O], BF16, tag="w")

            # ---- prologue DMAs ----
            # critical path first (shrink inputs on the sync queue)
            nc.sync.dma_start(out=a8_t[:], in_=a8[:])
            nc.gpsimd.dma_start(out=wc8_t[:], in_=wc8[:])
            nc.gpsimd.dma_start(out=ba_t[:], in_=bias_arr[:])
            for s in range(3):
                for tt in range(ntt):
                    nc.gpsimd.dma_start(
                        out=st_oh[s][:, tt, 1, :],
                        in_=oh8[:, tt * NT:(tt + 1) * NT])
            # W resident: one DMA per k-tile, on the gpsimd queue so the
            # sync queue stays free for per-tile x8/x streams
            for k in range(NKT):
                nc.gpsimd.dma_start(out=w_t[:, k, :], in_=wres[:, k, :])

            # ---- shrink pre-pass: st = mask/64 * (x8 @ (8A)^T) ----
            for tt in range(ntt):
                x8_t = x8p.tile([128, NQP, 2, NT], FP8, tag="x8",
                                name=f"x8_{tt}")
                nc.sync.dma_start(
                    out=x8_t[:], in_=x8[:, :, :, tt * NT:(tt + 1) * NT])
                mk_t = mkp.tile([128, NT], FP8, tag="mk")
                nc.sync.dma_start(
                    out=mk_t[:], in_=mask8[:, tt * NT:(tt + 1) * NT])
                for s in range(3):
                    ps = psm.tile([128, NT], F32, tag="ps")
                    for q in range(NQP):
                        nc.tensor.matmul(
                            ps[:],
                            a8_t[:, q, :, s * 128:(s + 1) * 128],
                            x8_t[:, q, :, :],
                            start=(q == 0), stop=(q == NQP - 1),
                            perf_mode=DR,
                        )
                    nc.vector.tensor_mul(
                        st_oh[s][:, tt, 0, :], ps[:], mk_t[:])

            # ---- main pass: single sweep over tokens, W resident ----
            for tt in range(ntt):
                x_t = xp.tile([128, NKT, NT], BF16, tag="x", name=f"x_{tt}")
                nc.sync.dma_start(
                    out=x_t[:], in_=xb[:, :, tt * NT:(tt + 1) * NT])
                for p in range(NBLK // 8):
                    # fused lora expand + lora bias (fp8 DoubleRow) for
                    # all 8 blocks first: one dtype switch per group
                    pss = []
                    for b in range(8):
                        j = p * 8 + b
                        s = 0 if j < QS // 128 else (
                            1 if j < (QS + KVS) // 128 else 2)
                        ps = psm.tile([128, NT], F32, tag="ps",
                                      name=f"ps{j}_{tt}")
                        pss.append(ps)
                        nc.tensor.matmul(
                            ps[:],
                            wc8_t[:, j, :, :],
                            st_oh[s][:, tt, :, :],
                            start=True, stop=False,
                            perf_mode=DR, skip_group_check=True,
                        )
                    for b in range(8):
                        j = p * 8 + b
                        for k in range(NKT):
                            nc.tensor.matmul(
                                pss[b][:],
                                w_t[:, k, j * 128:(j + 1) * 128],
                                x_t[:, k, :],
                                start=False, stop=(k == NKT - 1),
                                skip_group_check=True,
                            )
                        o_t = op.tile([128, NT], BF16, tag="o")
                        nc.vector.tensor_scalar_add(
                            o_t[:], pss[b][:], ba_t[:, j:j + 1])
                        nc.gpsimd.dma_start(
                            out=outT[j * 128:(j + 1) * 128,
                                     tt * NT:(tt + 1) * NT],
                            in_=o_t[:],
                        )
    nc.compile()
    return nc


_nc_cache = {}


def _get_program(tc_tokens=TC):
    if tc_tokens not in _nc_cache:
        _nc_cache[tc_tokens] = build_program(tc_tokens)
    return _nc_cache[tc_tokens]


def make_in_maps(x, W_qkv, bias_qkv, lora_a_q, lora_a_k, lora_a_v,
                 lora_b_q, lora_b_k, lora_b_v,
                 lora_bias_q, lora_bias_k, lora_bias_v,
                 token_lora_indices, ncores=NCORES):
    x = np.asarray(x, np.float32)
    idx = np.asarray(token_lora_indices).astype(np.int64)
    tc_tokens = x.shape[0] // ncores

    # W resident: [128, NKT, O], p-major over the contraction dim
    wres = np.ascontiguousarray(
        np.asarray(W_qkv, np.float32).T.reshape(NKT, 128, O)
        .transpose(1, 0, 2)).astype(BF16NP)

    # lora A stacked (q|k|v) -> [128, NQP, 2, 384], scaled x8 for fp8
    a_stack = np.concatenate([
        np.asarray(lora_a_q, np.float32).reshape(L * R, D),
        np.asarray(lora_a_k, np.float32).reshape(L * R, D),
        np.asarray(lora_a_v, np.float32).reshape(L * R, D)], axis=0)
    a8 = np.ascontiguousarray(
        (a_stack.T * 8.0).reshape(NQP, 2, 128, 384)
        .transpose(2, 0, 1, 3)).astype(FP8NP)

    # fused expand weights: plane 0 = 8*bcomb, plane 1 = 8*biasL
    # zero-padded to 128 partitions
    bcomb = np.concatenate([
        np.asarray(lora_b_q, np.float32).transpose(0, 2, 1).reshape(L * R, QS),
        np.asarray(lora_b_k, np.float32).transpose(0, 2, 1).reshape(L * R, KVS),
        np.asarray(lora_b_v, np.float32).transpose(0, 2, 1).reshape(L * R, KVS)],
        axis=1)                                     # (128, O)
    biasL = np.concatenate([
        np.asarray(lora_bias_q, np.float32),
        np.asarray(lora_bias_k, np.float32),
        np.asarray(lora_bias_v, np.float32)], axis=1)  # (L, O)
    biasLpad = np.zeros((128, O), np.float32)
    biasLpad[:L] = biasL
    wc8 = np.empty((128, NBLK, 2, 128), np.float32)
    wc8[:, :, 0, :] = (8.0 * bcomb).reshape(128, NBLK, 128)
    wc8[:, :, 1, :] = (8.0 * biasLpad).reshape(128, NBLK, 128)
    wc8 = wc8.astype(FP8NP)

    bias_arr = np.ascontiguousarray(
        np.asarray(bias_qkv, np.float32).reshape(NBLK, 128).T)
    lane = np.arange(128) // R

    xT_all = np.asarray(x, np.float32).T            # (D, T)
    in_maps = []
    for c in range(ncores):
        sl = slice(c * tc_tokens, (c + 1) * tc_tokens)
        idx_c = idx[sl]
        xT = xT_all[:, sl]                          # (D, tc)
        ohpad = np.zeros((128, tc_tokens), np.float32)
        ohpad[:L] = (idx_c[None, :] == np.arange(L)[:, None]) / 8.0
        in_maps.append({
            "xb": np.ascontiguousarray(
                xT.reshape(NKT, 128, tc_tokens).transpose(1, 0, 2)
            ).astype(BF16NP),
            "x8": np.ascontiguousarray(
                xT.reshape(NQP, 2, 128, tc_tokens).transpose(2, 0, 1, 3)
            ).astype(FP8NP),
            "wres": wres,
            "a8": a8,
            "wc8": wc8,
            "oh8": ohpad.astype(FP8NP),
            "mask8": ((idx_c[None, :] == lane[:, None]) / 64.0).astype(FP8NP),
            "bias_arr": bias_arr,
        })
    return in_maps, tc_tokens


def kernel(x, W_qkv, bias_qkv, lora_a_q, lora_a_k, lora_a_v,
           lora_b_q, lora_b_k, lora_b_v,
           lora_bias_q, lora_bias_k, lora_bias_v,
           token_lora_indices):
    in_maps, tc_tokens = make_in_maps(
        x, W_qkv, bias_qkv, lora_a_q, lora_a_k, lora_a_v,
        lora_b_q, lora_b_k, lora_b_v,
        lora_bias_q, lora_bias_k, lora_bias_v, token_lora_indices)
    nc = _get_program(tc_tokens)
    res = run_bass_kernel_spmd(nc, in_maps, list(range(NCORES)))
    out = np.empty((T, O), np.float32)
    for c in range(NCORES):
        out[c * tc_tokens:(c + 1) * tc_tokens] = \
            res.results[c]["outT"].T.astype(np.float32)
    return out


# revision 8
# speedup vs baseline: 1.2525x; 1.0309x over previous
"""MergedQKVParallelLinearWithLora on 8 TRN2 NeuronCores.

Strategy: token-parallel (data-parallel) across the 8 cores — each core
computes 4096 tokens of the full (T=32768, O=3072) output.

v2 layout (vs the f32r baseline at ~1005us):
  - main GEMM in bf16 (same 1 cyc/row PE rate as f32r, but LDWEIGHTS is
    4x shorter so it fully shadow-loads under the previous matmul)
  - W fully SBUF-resident (12.5MB bf16), single pass over x: per-core
    HBM traffic drops ~164MB -> ~66MB, no pass-boundary DMA stalls
  - lora shrink in fp8e4 DoubleRow: 2 k-tiles per matmul instruction
    (half the instructions of the f32r shrink)
  - lora expand + lora bias fused into ONE fp8e4 DoubleRow matmul per
    output block: plane 0 = bcomb x st, plane 1 = biasL x one-hot
    (zero-padded to 128 partitions)
  - fp8 scaling keeps everything in e4m3 normal range: A,B,biasL are
    scaled x8 host-side; the shrink mask is 1/64 (undoes A's x8 and
    pre-divides by 8), the one-hot is 1/8
  - output stored bf16 (halves writeback; host converts back to f32)

Per-core PE work: 192 shrink DR + 192 fused-expand DR + 3072 bf16 main
matmuls of 512 moving rows each ~= 755us at 2.4GHz / 1 cyc/row.
"""

import numpy as np
import ml_dtypes

import concourse.mybir as mybir
import concourse.tile as tile
from concourse import bacc
from concourse.bass_utils import run_bass_kernel_spmd

T, D, QS, KVS, L, R = 32768, 2048, 2048, 512, 8, 16
O = QS + 2 * KVS          # 3072
NCORES = 8
TC = T // NCORES          # 4096 tokens per core
NT = 512                  # tokens per tile (matmul moving dim)
NKT = D // 128            # 16 contraction k-tiles
NQP = NKT // 2            # 8 DoubleRow k-tile pairs
NBLK = O // 128           # 24 output-channel blocks

F32 = mybir.dt.float32
BF16 = mybir.dt.bfloat16
FP8 = mybir.dt.float8e4
DR = mybir.MatmulPerfMode.DoubleRow
BF16NP = ml_dtypes.bfloat16
FP8NP = ml_dtypes.float8_e4m3


def build_program(tc_tokens=TC):
    ntt = tc_tokens // NT
    nc = bacc.Bacc(None, target_bir_lowering=False, debug=False)

    ntt_ = tc_tokens // NT
    xb = nc.dram_tensor("xb", [ntt_, 128, NKT, NT], BF16, kind="ExternalInput")
    # per-token-tile shrink input: 16 fp8 x k-planes + the shrink mask as
    # plane 17, so one large-line DMA delivers everything the tile needs
    x8m = nc.dram_tensor("x8m", [ntt_, 128, NQP * 2 + 1, NT], FP8,
                         kind="ExternalInput")
    wres = nc.dram_tensor("wres", [128, NKT, O], BF16, kind="ExternalInput")
    a8 = nc.dram_tensor("a8", [128, NQP, 2, 384], FP8, kind="ExternalInput")
    wc8 = nc.dram_tensor("wc8", [128, NBLK, 2, 128], FP8, kind="ExternalInput")
    oh8 = nc.dram_tensor("oh8", [128, tc_tokens], FP8, kind="ExternalInput")
    bias_arr = nc.dram_tensor("bias_arr", [128, NBLK], F32, kind="ExternalInput")
    outT = nc.dram_tensor("outT", [O, tc_tokens], BF16, kind="ExternalOutput")

    with tile.TileContext(nc) as tc:
        with tc.tile_pool(name="const", bufs=1) as const, \
             tc.tile_pool(name="xp", bufs=2) as xp, \
             tc.tile_pool(name="x8p", bufs=2) as x8p, \
             tc.tile_pool(name="psm", bufs=8, space="PSUM") as psm, \
             tc.tile_pool(name="op", bufs=4) as op:
            # st_oh[s]: interleaved moving operand for the fused expand
            # matmul: [:, tt, 0, :] = masked shrink (s/8, fp8),
            # [:, tt, 1, :] = padded one-hot / 8
            st_oh = [const.tile([128, ntt, 2, NT], FP8, tag=f"st{s}",
                                name=f"st{s}") for s in range(3)]
            a8_t = const.tile([128, NQP, 2, 384], FP8, tag="a8")
            wc8_t = const.tile([128, NBLK, 2, 128], FP8, tag="wc8")
            ba_t = const.tile([128, NBLK], F32, tag="ba")
            w_t = const.tile([128, NKT, O], BF16, tag="w")

            # ---- prologue DMAs ----
            # critical path first (shrink inputs on the sync queue)
            nc.sync.dma_start(out=a8_t[:], in_=a8[:])
            nc.gpsimd.dma_start(out=wc8_t[:], in_=wc8[:])
            nc.gpsimd.dma_start(out=ba_t[:], in_=bias_arr[:])
            for s in range(3):
                for tt in range(ntt):
                    nc.gpsimd.dma_start(
                        out=st_oh[s][:, tt, 1, :],
                        in_=oh8[:, tt * NT:(tt + 1) * NT])
            # W resident: one DMA per k-tile, on the gpsimd queue so the
            # sync queue stays free for per-tile x8/x streams
            for k in range(NKT):
                nc.gpsimd.dma_start(out=w_t[:, k, :], in_=wres[:, k, :])

            # ---- shrink pre-pass: st = mask/64 * (x8 @ (8A)^T) ----
            for tt in range(ntt):
                x8_t = x8p.tile([128, NQP * 2 + 1, NT], FP8, tag="x8",
                                name=f"x8_{tt}")
                nc.sync.dma_start(out=x8_t[:], in_=x8m[tt])
                for s in range(3):
                    ps = psm.tile([128, NT], F32, tag="ps")
                    for q in range(NQP):
                        nc.tensor.matmul(
                            ps[:],
                            a8_t[:, q, :, s * 128:(s + 1) * 128],
                            x8_t[:, 2 * q:2 * q + 2, :],
                            start=(q == 0), stop=(q == NQP - 1),
                            perf_mode=DR,
                        )
                    nc.vector.tensor_mul(
                        st_oh[s][:, tt, 0, :], ps[:], x8_t[:, NQP * 2, :])

            # ---- main pass: single sweep over tokens, W resident ----
            for tt in range(ntt):
                x_t = xp.tile([128, NKT, NT], BF16, tag="x", name=f"x_{tt}")
                nc.sync.dma_start(out=x_t[:], in_=xb[tt])
                for p in range(NBLK // 8):
                    # fused lora expand + lora bias (fp8 DoubleRow) for
                    # all 8 blocks first: one dtype switch per group
                    pss = []
                    for b in range(8):
                        j = p * 8 + b
                        s = 0 if j < QS // 128 else (
                            1 if j < (QS + KVS) // 128 else 2)
                        ps = psm.tile([128, NT], F32, tag="ps",
                                      name=f"ps{j}_{tt}")
                        pss.append(ps)
                        nc.tensor.matmul(
                            ps[:],
                            wc8_t[:, j, :, :],
                            st_oh[s][:, tt, :, :],
                            start=True, stop=False,
                            perf_mode=DR, skip_group_check=True,
                        )
                    for b in range(8):
                        j = p * 8 + b
                        for k in range(NKT):
                            nc.tensor.matmul(
                                pss[b][:],
                                w_t[:, k, j * 128:(j + 1) * 128],
                                x_t[:, k, :],
                                start=False, stop=(k == NKT - 1),
                                skip_group_check=True,
                            )
                        o_t = op.tile([128, NT], BF16, tag="o")
                        nc.vector.tensor_scalar_add(
                            o_t[:], pss[b][:], ba_t[:, j:j + 1])
                        nc.gpsimd.dma_start(
                            out=outT[j * 128:(j + 1) * 128,
                                     tt * NT:(tt + 1) * NT],
                            in_=o_t[:],
                        )
    nc.compile()
    return nc


_nc_cache = {}


def _get_program(tc_tokens=TC):
    if tc_tokens not in _nc_cache:
        _nc_cache[tc_tokens] = build_program(tc_tokens)
    return _nc_cache[tc_tokens]


def make_in_maps(x, W_qkv, bias_qkv, lora_a_q, lora_a_k, lora_a_v,
                 lora_b_q, lora_b_k, lora_b_v,
                 lora_bias_q, lora_bias_k, lora_bias_v,
                 token_lora_indices, ncores=NCORES):
    x = np.asarray(x, np.float32)
    idx = np.asarray(token_lora_indices).astype(np.int64)
    tc_tokens = x.shape[0] // ncores

    # W resident: [128, NKT, O], p-major over the contraction dim
    wres = np.ascontiguousarray(
        np.asarray(W_qkv, np.float32).T.reshape(NKT, 128, O)
        .transpose(1, 0, 2)).astype(BF16NP)

    # lora A stacked (q|k|v) -> [128, NQP, 2, 384], scaled x8 for fp8
    a_stack = np.concatenate([
        np.asarray(lora_a_q, np.float32).reshape(L * R, D),
        np.asarray(lora_a_k, np.float32).reshape(L * R, D),
        np.asarray(lora_a_v, np.float32).reshape(L * R, D)], axis=0)
    a8 = np.ascontiguousarray(
        (a_stack.T * 8.0).reshape(NQP, 2, 128, 384)
        .transpose(2, 0, 1, 3)).astype(FP8NP)

    # fused expand weights: plane 0 = 8*bcomb, plane 1 = 8*biasL
    # zero-padded to 128 partitions
    bcomb = np.concatenate([
        np.asarray(lora_b_q, np.float32).transpose(0, 2, 1).reshape(L * R, QS),
        np.asarray(lora_b_k, np.float32).transpose(0, 2, 1).reshape(L * R, KVS),
        np.asarray(lora_b_v, np.float32).transpose(0, 2, 1).reshape(L * R, KVS)],
        axis=1)                                     # (128, O)
    biasL = np.concatenate([
        np.asarray(lora_bias_q, np.float32),
        np.asarray(lora_bias_k, np.float32),
        np.asarray(lora_bias_v, np.float32)], axis=1)  # (L, O)
    biasLpad = np.zeros((128, O), np.float32)
    biasLpad[:L] = biasL
    wc8 = np.empty((128, NBLK, 2, 128), np.float32)
    wc8[:, :, 0, :] = (8.0 * bcomb).reshape(128, NBLK, 128)
    wc8[:, :, 1, :] = (8.0 * biasLpad).reshape(128, NBLK, 128)
    wc8 = wc8.astype(FP8NP)

    bias_arr = np.ascontiguousarray(
        np.asarray(bias_qkv, np.float32).reshape(NBLK, 128).T)
    lane = np.arange(128) // R

    xT_all = np.asarray(x, np.float32).T            # (D, T)
    ntt = tc_tokens // NT
    in_maps = []
    for c in range(ncores):
        sl = slice(c * tc_tokens, (c + 1) * tc_tokens)
        idx_c = idx[sl]
        xT = xT_all[:, sl]                          # (D, tc)
        ohpad = np.zeros((128, tc_tokens), np.float32)
        ohpad[:L] = (idx_c[None, :] == np.arange(L)[:, None]) / 8.0
        # xb[tt, p, k, n] = x^T[k*128+p, tt*NT+n]
        xb = np.ascontiguousarray(
            xT.reshape(NKT, 128, ntt, NT).transpose(2, 1, 0, 3)
        ).astype(BF16NP)
        # x8m[tt, p, 2q+i, n] = x^T[q*256+i*128+p, tt*NT+n]; plane 16 = mask
        x8m = np.empty((ntt, 128, NQP * 2 + 1, NT), FP8NP)
        x8m[:, :, :NQP * 2, :] = (
            xT.reshape(NQP * 2, 128, ntt, NT).transpose(2, 1, 0, 3)
        ).astype(FP8NP)
        x8m[:, :, NQP * 2, :] = (
            ((idx_c[None, :] == lane[:, None]) / 64.0).astype(FP8NP)
            .reshape(128, ntt, NT).transpose(1, 0, 2))
        in_maps.append({
            "xb": xb,
            "x8m": x8m,
            "wres": wres,
            "a8": a8,
            "wc8": wc8,
            "oh8": ohpad.astype(FP8NP),
            "bias_arr": bias_arr,
        })
    return in_maps, tc_tokens


def kernel(x, W_qkv, bias_qkv, lora_a_q, lora_a_k, lora_a_v,
           lora_b_q, lora_b_k, lora_b_v,
           lora_bias_q, lora_bias_k, lora_bias_v,
           token_lora_indices):
    in_maps, tc_tokens = make_in_maps(
        x, W_qkv, bias_qkv, lora_a_q, lora_a_k, lora_a_v,
        lora_b_q, lora_b_k, lora_b_v,
        lora_bias_q, lora_bias_k, lora_bias_v, token_lora_indices)
    nc = _get_program(tc_tokens)
    res = run_bass_kernel_spmd(nc, in_maps, list(range(NCORES)))
    out = np.empty((T, O), np.float32)
    for c in range(NCORES):
        out[c * tc_tokens:(c + 1) * tc_tokens] = \
            res.results[c]["outT"].T.astype(np.float32)
    return out


# revision 9
# speedup vs baseline: 1.2588x; 1.0051x over previous
"""MergedQKVParallelLinearWithLora on 8 TRN2 NeuronCores.

Strategy: token-parallel (data-parallel) across the 8 cores — each core
computes 4096 tokens of the full (T=32768, O=3072) output.

v2 layout (vs the f32r baseline at ~1005us):
  - main GEMM in bf16 (same 1 cyc/row PE rate as f32r, but LDWEIGHTS is
    4x shorter so it fully shadow-loads under the previous matmul)
  - W fully SBUF-resident (12.5MB bf16), single pass over x: per-core
    HBM traffic drops ~164MB -> ~66MB, no pass-boundary DMA stalls
  - lora shrink in fp8e4 DoubleRow: 2 k-tiles per matmul instruction
    (half the instructions of the f32r shrink)
  - lora expand + lora bias fused into ONE fp8e4 DoubleRow matmul per
    output block: plane 0 = bcomb x st, plane 1 = biasL x one-hot
    (zero-padded to 128 partitions)
  - fp8 scaling keeps everything in e4m3 normal range: A,B,biasL are
    scaled x8 host-side; the shrink mask is 1/64 (undoes A's x8 and
    pre-divides by 8), the one-hot is 1/8
  - output stored bf16 (halves writeback; host converts back to f32)

Per-core PE work: 192 shrink DR + 192 fused-expand DR + 3072 bf16 main
matmuls of 512 moving rows each ~= 755us at 2.4GHz / 1 cyc/row.
"""

import numpy as np
import ml_dtypes

import concourse.mybir as mybir
import concourse.tile as tile
from concourse import bacc
from concourse.bass_utils import run_bass_kernel_spmd

T, D, QS, KVS, L, R = 32768, 2048, 2048, 512, 8, 16
O = QS + 2 * KVS          # 3072
NCORES = 8
TC = T // NCORES          # 4096 tokens per core
NT = 512                  # tokens per tile (matmul moving dim)
NKT = D // 128            # 16 contraction k-tiles
NQP = NKT // 2            # 8 DoubleRow k-tile pairs
NBLK = O // 128           # 24 output-channel blocks

F32 = mybir.dt.float32
BF16 = mybir.dt.bfloat16
FP8 = mybir.dt.float8e4
DR = mybir.MatmulPerfMode.DoubleRow
BF16NP = ml_dtypes.bfloat16
FP8NP = ml_dtypes.float8_e4m3


def build_program(tc_tokens=TC):
    ntt = tc_tokens // NT
    nc = bacc.Bacc(None, target_bir_lowering=False, debug=False)

    ntt_ = tc_tokens // NT
    xb = nc.dram_tensor("xb", [ntt_, 128, NKT, NT], BF16, kind="ExternalInput")
    # per-token-tile shrink input: 16 fp8 x k-planes + the shrink mask as
    # plane 17, so one large-line DMA delivers everything the tile needs
    x8m = nc.dram_tensor("x8m", [ntt_, 128, NQP * 2 + 1, NT], FP8,
                         kind="ExternalInput")
    wres = nc.dram_tensor("wres", [128, NKT, O], BF16, kind="ExternalInput")
    a8 = nc.dram_tensor("a8", [128, NQP, 2, 384], FP8, kind="ExternalInput")
    wc8 = nc.dram_tensor("wc8", [128, NBLK, 2, 128], FP8, kind="ExternalInput")
    oh8 = nc.dram_tensor("oh8", [128, tc_tokens], FP8, kind="ExternalInput")
    bias_arr = nc.dram_tensor("bias_arr", [128, NBLK], F32, kind="ExternalInput")
    outT = nc.dram_tensor("outT", [O, tc_tokens], BF16, kind="ExternalOutput")

    with tile.TileContext(nc) as tc:
        with tc.tile_pool(name="const", bufs=1) as const, \
             tc.tile_pool(name="xp", bufs=2) as xp, \
             tc.tile_pool(name="x8p", bufs=2) as x8p, \
             tc.tile_pool(name="psm", bufs=8, space="PSUM") as psm, \
             tc.tile_pool(name="op", bufs=4) as op:
            # st_oh[s]: interleaved moving operand for the fused expand
            # matmul: [:, tt, 0, :] = masked shrink (s/8, fp8),
            # [:, tt, 1, :] = padded one-hot / 8
            st_oh = [const.tile([128, ntt, 2, NT], FP8, tag=f"st{s}",
                                name=f"st{s}") for s in range(3)]
            a8_t = const.tile([128, NQP, 2, 384], FP8, tag="a8")
            wc8_t = const.tile([128, NBLK, 2, 128], FP8, tag="wc8")
            ba_t = const.tile([128, NBLK], F32, tag="ba")
            w_t = const.tile([128, NKT, O], BF16, tag="w")

            # ---- prologue DMAs ----
            # shrink inputs on the sync queue (critical path)
            nc.sync.dma_start(out=a8_t[:], in_=a8[:])
            nc.gpsimd.dma_start(out=wc8_t[:], in_=wc8[:])
            nc.gpsimd.dma_start(out=ba_t[:], in_=bias_arr[:])
            for s in range(3):
                for tt in range(ntt):
                    nc.gpsimd.dma_start(
                        out=st_oh[s][:, tt, 1, :],
                        in_=oh8[:, tt * NT:(tt + 1) * NT])
            # W resident, streamed on the (otherwise idle) Activation
            # queue in consumption order: p-group-major chunks so the
            # first 8 output blocks are ready before the first main group
            for p in range(NBLK // 8):
                for k in range(NKT):
                    nc.scalar.dma_start(
                        out=w_t[:, k, p * 1024:(p + 1) * 1024],
                        in_=wres[:, k, p * 1024:(p + 1) * 1024])

            def shrink(tt):
                # st = mask/64 * (x8 @ (8A)^T), fp8 DoubleRow
                x8_t = x8p.tile([128, NQP * 2 + 1, NT], FP8, tag="x8",
                                name=f"x8_{tt}")
                nc.sync.dma_start(out=x8_t[:], in_=x8m[tt])
                for s in range(3):
                    ps = psm.tile([128, NT], F32, tag="ps")
                    for q in range(NQP):
                        nc.tensor.matmul(
                            ps[:],
                            a8_t[:, q, :, s * 128:(s + 1) * 128],
                            x8_t[:, 2 * q:2 * q + 2, :],
                            start=(q == 0), stop=(q == NQP - 1),
                            perf_mode=DR,
                        )
                    nc.vector.tensor_mul(
                        st_oh[s][:, tt, 0, :], ps[:], x8_t[:, NQP * 2, :])

            def load_x(tt):
                x_t = xp.tile([128, NKT, NT], BF16, tag="x", name=f"x_{tt}")
                nc.sync.dma_start(out=x_t[:], in_=xb[tt])
                return x_t

            def main_group(tt, p, x_t):
                # fused lora expand + lora bias (fp8 DoubleRow) for all
                # 8 blocks first: one dtype switch per group
                pss = []
                for b in range(8):
                    j = p * 8 + b
                    s = 0 if j < QS // 128 else (
                        1 if j < (QS + KVS) // 128 else 2)
                    ps = psm.tile([128, NT], F32, tag="ps",
                                  name=f"ps{j}_{tt}")
                    pss.append(ps)
                    nc.tensor.matmul(
                        ps[:],
                        wc8_t[:, j, :, :],
                        st_oh[s][:, tt, :, :],
                        start=True, stop=False,
                        perf_mode=DR, skip_group_check=True,
                    )
                for b in range(8):
                    j = p * 8 + b
                    for k in range(NKT):
                        nc.tensor.matmul(
                            pss[b][:],
                            w_t[:, k, j * 128:(j + 1) * 128],
                            x_t[:, k, :],
                            start=False, stop=(k == NKT - 1),
                            skip_group_check=True,
                        )
                    o_t = op.tile([128, NT], BF16, tag="o")
                    nc.vector.tensor_scalar_add(
                        o_t[:], pss[b][:], ba_t[:, j:j + 1])
                    nc.gpsimd.dma_start(
                        out=outT[j * 128:(j + 1) * 128,
                                 tt * NT:(tt + 1) * NT],
                        in_=o_t[:],
                    )

            # ---- software-pipelined sweep: shrink(tt+1) hides inside
            # main(tt) so the PE never waits on the x8 stream, and main
            # work starts as soon as the first shrink tile is done ----
            shrink(0)
            x_t = load_x(0)
            for tt in range(ntt):
                main_group(tt, 0, x_t)
                main_group(tt, 1, x_t)
                if tt + 1 < ntt:
                    shrink(tt + 1)
                    x_next = load_x(tt + 1)
                else:
                    x_next = None
                main_group(tt, 2, x_t)
                x_t = x_next
    nc.compile()
    return nc


_nc_cache = {}


def _get_program(tc_tokens=TC):
    if tc_tokens not in _nc_cache:
        _nc_cache[tc_tokens] = build_program(tc_tokens)
    return _nc_cache[tc_tokens]


def make_in_maps(x, W_qkv, bias_qkv, lora_a_q, lora_a_k, lora_a_v,
                 lora_b_q, lora_b_k, lora_b_v,
                 lora_bias_q, lora_bias_k, lora_bias_v,
                 token_lora_indices, ncores=NCORES):
    x = np.asarray(x, np.float32)
    idx = np.asarray(token_lora_indices).astype(np.int64)
    tc_tokens = x.shape[0] // ncores

    # W resident: [128, NKT, O], p-major over the contraction dim
    wres = np.ascontiguousarray(
        np.asarray(W_qkv, np.float32).T.reshape(NKT, 128, O)
        .transpose(1, 0, 2)).astype(BF16NP)

    # lora A stacked (q|k|v) -> [128, NQP, 2, 384], scaled x8 for fp8
    a_stack = np.concatenate([
        np.asarray(lora_a_q, np.float32).reshape(L * R, D),
        np.asarray(lora_a_k, np.float32).reshape(L * R, D),
        np.asarray(lora_a_v, np.float32).reshape(L * R, D)], axis=0)
    a8 = np.ascontiguousarray(
        (a_stack.T * 8.0).reshape(NQP, 2, 128, 384)
        .transpose(2, 0, 1, 3)).astype(FP8NP)

    # fused expand weights: plane 0 = 8*bcomb, plane 1 = 8*biasL
    # zero-padded to 128 partitions
    bcomb = np.concatenate([
        np.asarray(lora_b_q, np.float32).transpose(0, 2, 1).reshape(L * R, QS),
        np.asarray(lora_b_k, np.float32).transpose(0, 2, 1).reshape(L * R, KVS),
        np.asarray(lora_b_v, np.float32).transpose(0, 2, 1).reshape(L * R, KVS)],
        axis=1)                                     # (128, O)
    biasL = np.concatenate([
        np.asarray(lora_bias_q, np.float32),
        np.asarray(lora_bias_k, np.float32),
        np.asarray(lora_bias_v, np.float32)], axis=1)  # (L, O)
    biasLpad = np.zeros((128, O), np.float32)
    biasLpad[:L] = biasL
    wc8 = np.empty((128, NBLK, 2, 128), np.float32)
    wc8[:, :, 0, :] = (8.0 * bcomb).reshape(128, NBLK, 128)
    wc8[:, :, 1, :] = (8.0 * biasLpad).reshape(128, NBLK, 128)
    wc8 = wc8.astype(FP8NP)

    bias_arr = np.ascontiguousarray(
        np.asarray(bias_qkv, np.float32).reshape(NBLK, 128).T)
    lane = np.arange(128) // R

    xT_all = np.asarray(x, np.float32).T            # (D, T)
    ntt = tc_tokens // NT
    in_maps = []
    for c in range(ncores):
        sl = slice(c * tc_tokens, (c + 1) * tc_tokens)
        idx_c = idx[sl]
        xT = xT_all[:, sl]                          # (D, tc)
        ohpad = np.zeros((128, tc_tokens), np.float32)
        ohpad[:L] = (idx_c[None, :] == np.arange(L)[:, None]) / 8.0
        # xb[tt, p, k, n] = x^T[k*128+p, tt*NT+n]
        xb = np.ascontiguousarray(
            xT.reshape(NKT, 128, ntt, NT).transpose(2, 1, 0, 3)
        ).astype(BF16NP)
        # x8m[tt, p, 2q+i, n] = x^T[q*256+i*128+p, tt*NT+n]; plane 16 = mask
        x8m = np.empty((ntt, 128, NQP * 2 + 1, NT), FP8NP)
        x8m[:, :, :NQP * 2, :] = (
            xT.reshape(NQP * 2, 128, ntt, NT).transpose(2, 1, 0, 3)
        ).astype(FP8NP)
        x8m[:, :, NQP * 2, :] = (
            ((idx_c[None, :] == lane[:, None]) / 64.0).astype(FP8NP)
            .reshape(128, ntt, NT).transpose(1, 0, 2))
        in_maps.append({
            "xb": xb,
            "x8m": x8m,
            "wres": wres,
            "a8": a8,
            "wc8": wc8,
            "oh8": ohpad.astype(FP8NP),
            "bias_arr": bias_arr,
        })
    return in_maps, tc_tokens


def kernel(x, W_qkv, bias_qkv, lora_a_q, lora_a_k, lora_a_v,
           lora_b_q, lora_b_k, lora_b_v,
           lora_bias_q, lora_bias_k, lora_bias_v,
           token_lora_indices):
    in_maps, tc_tokens = make_in_maps(
        x, W_qkv, bias_qkv, lora_a_q, lora_a_k, lora_a_v,
        lora_b_q, lora_b_k, lora_b_v,
        lora_bias_q, lora_bias_k, lora_bias_v, token_lora_indices)
    nc = _get_program(tc_tokens)
    res = run_bass_kernel_spmd(nc, in_maps, list(range(NCORES)))
    out = np.empty((T, O), np.float32)
    for c in range(NCORES):
        out[c * tc_tokens:(c + 1) * tc_tokens] = \
            res.results[c]["outT"].T.astype(np.float32)
    return out


# revision 13
# speedup vs baseline: 1.2656x; 1.0054x over previous
"""MergedQKVParallelLinearWithLora on 8 TRN2 NeuronCores.

Strategy: token-parallel (data-parallel) across the 8 cores — each core
computes 4096 tokens of the full (T=32768, O=3072) output.

v2 layout (vs the f32r baseline at ~1005us):
  - main GEMM in bf16 (same 1 cyc/row PE rate as f32r, but LDWEIGHTS is
    4x shorter so it fully shadow-loads under the previous matmul)
  - W fully SBUF-resident (12.5MB bf16), single pass over x: per-core
    HBM traffic drops ~164MB -> ~66MB, no pass-boundary DMA stalls
  - lora shrink in fp8e4 DoubleRow: 2 k-tiles per matmul instruction
    (half the instructions of the f32r shrink)
  - lora expand + lora bias fused into ONE fp8e4 DoubleRow matmul per
    output block: plane 0 = bcomb x st, plane 1 = biasL x one-hot
    (zero-padded to 128 partitions)
  - fp8 scaling keeps everything in e4m3 normal range: A,B,biasL are
    scaled x8 host-side; the shrink mask is 1/64 (undoes A's x8 and
    pre-divides by 8), the one-hot is 1/8
  - output stored bf16 (halves writeback; host converts back to f32)

Per-core PE work: 192 shrink DR + 192 fused-expand DR + 3072 bf16 main
matmuls of 512 moving rows each ~= 755us at 2.4GHz / 1 cyc/row.
"""

import numpy as np
import ml_dtypes

import concourse.mybir as mybir
import concourse.tile as tile
from concourse import bacc
from concourse.bass_utils import run_bass_kernel_spmd

T, D, QS, KVS, L, R = 32768, 2048, 2048, 512, 8, 16
O = QS + 2 * KVS          # 3072
NCORES = 8
TC = T // NCORES          # 4096 tokens per core
NT = 512                  # tokens per tile (matmul moving dim)
NKT = D // 128            # 16 contraction k-tiles
NQP = NKT // 2            # 8 DoubleRow k-tile pairs
NBLK = O // 128           # 24 output-channel blocks

F32 = mybir.dt.float32
BF16 = mybir.dt.bfloat16
FP8 = mybir.dt.float8e4
DR = mybir.MatmulPerfMode.DoubleRow
BF16NP = ml_dtypes.bfloat16
FP8NP = ml_dtypes.float8_e4m3


def build_program(tc_tokens=TC):
    ntt = tc_tokens // NT
    nc = bacc.Bacc(None, target_bir_lowering=False, debug=False)

    ntt_ = tc_tokens // NT
    xb = nc.dram_tensor("xb", [ntt_, 128, NKT, NT], BF16, kind="ExternalInput")
    # per-token-tile shrink input: 16 fp8 x k-planes + the shrink mask as
    # plane 17, so one large-line DMA delivers everything the tile needs
    x8m = nc.dram_tensor("x8m", [ntt_, 128, NQP * 2 + 1, NT], FP8,
                         kind="ExternalInput")
    wres = nc.dram_tensor("wres", [128, NKT, O], BF16, kind="ExternalInput")
    a8 = nc.dram_tensor("a8", [128, NQP, 2, 384], FP8, kind="ExternalInput")
    wc8 = nc.dram_tensor("wc8", [128, NBLK, 2, 128], FP8, kind="ExternalInput")
    oh8 = nc.dram_tensor("oh8", [128, tc_tokens], FP8, kind="ExternalInput")
    bias_arr = nc.dram_tensor("bias_arr", [128, NBLK], F32, kind="ExternalInput")
    outT = nc.dram_tensor("outT", [O, tc_tokens], BF16, kind="ExternalOutput")

    with tile.TileContext(nc) as tc:
        with tc.tile_pool(name="const", bufs=1) as const, \
             tc.tile_pool(name="xp", bufs=2) as xp, \
             tc.tile_pool(name="x8p", bufs=2) as x8p, \
             tc.tile_pool(name="psm", bufs=8, space="PSUM") as psm, \
             tc.tile_pool(name="op", bufs=4) as op:
            # st_oh[s]: interleaved moving operand for the fused expand
            # matmul: [:, tt, 0, :] = masked shrink (s/8, fp8),
            # [:, tt, 1, :] = padded one-hot / 8
            st_oh = [const.tile([128, ntt, 2, NT], FP8, tag=f"st{s}",
                                name=f"st{s}") for s in range(3)]
            a8_t = const.tile([128, NQP, 2, 384], FP8, tag="a8")
            wc8_t = const.tile([128, NBLK, 2, 128], FP8, tag="wc8")
            ba_t = const.tile([128, NBLK], F32, tag="ba")
            w_t = const.tile([128, NKT, O], BF16, tag="w")

            # ---- prologue DMAs ----
            # shrink inputs on the sync queue (critical path)
            nc.sync.dma_start(out=a8_t[:], in_=a8[:])
            nc.gpsimd.dma_start(out=wc8_t[:], in_=wc8[:])
            nc.gpsimd.dma_start(out=ba_t[:], in_=bias_arr[:])
            # tt-major so tile 0/1's one-hot planes land before main(0)
            for tt in range(ntt):
                for s in range(3):
                    nc.gpsimd.dma_start(
                        out=st_oh[s][:, tt, 1, :],
                        in_=oh8[:, tt * NT:(tt + 1) * NT])
            # W resident, streamed on the (otherwise idle) Activation
            # queue in consumption order: p-group-major chunks so the
            # first 8 output blocks are ready before the first main group
            def load_w(p):
                for k in range(NKT):
                    nc.scalar.dma_start(
                        out=w_t[:, k, p * 1024:(p + 1) * 1024],
                        in_=wres[:, k, p * 1024:(p + 1) * 1024])

            def shrink(tt):
                # st = mask/64 * (x8 @ (8A)^T), fp8 DoubleRow
                x8_t = x8p.tile([128, NQP * 2 + 1, NT], FP8, tag="x8",
                                name=f"x8_{tt}")
                nc.sync.dma_start(out=x8_t[:], in_=x8m[tt])
                for s in range(3):
                    ps = psm.tile([128, NT], F32, tag="ps")
                    for q in range(NQP):
                        nc.tensor.matmul(
                            ps[:],
                            a8_t[:, q, :, s * 128:(s + 1) * 128],
                            x8_t[:, 2 * q:2 * q + 2, :],
                            start=(q == 0), stop=(q == NQP - 1),
                            perf_mode=DR,
                        )
                    nc.vector.tensor_mul(
                        st_oh[s][:, tt, 0, :], ps[:], x8_t[:, NQP * 2, :])

            def load_x(tt):
                x_t = xp.tile([128, NKT, NT], BF16, tag="x", name=f"x_{tt}")
                nc.scalar.dma_start(out=x_t[:], in_=xb[tt])
                return x_t

            def main_group(tt, p, x_t):
                # fused lora expand + lora bias (fp8 DoubleRow) for all
                # 8 blocks first: one dtype switch per group
                pss = []
                for b in range(8):
                    j = p * 8 + b
                    s = 0 if j < QS // 128 else (
                        1 if j < (QS + KVS) // 128 else 2)
                    ps = psm.tile([128, NT], F32, tag="ps",
                                  name=f"ps{j}_{tt}")
                    pss.append(ps)
                    nc.tensor.matmul(
                        ps[:],
                        wc8_t[:, j, :, :],
                        st_oh[s][:, tt, :, :],
                        start=True, stop=False,
                        perf_mode=DR, skip_group_check=True,
                    )
                for b in range(8):
                    j = p * 8 + b
                    for k in range(NKT):
                        nc.tensor.matmul(
                            pss[b][:],
                            w_t[:, k, j * 128:(j + 1) * 128],
                            x_t[:, k, :],
                            start=False, stop=(k == NKT - 1),
                            skip_group_check=True,
                        )
                    o_t = op.tile([128, NT], BF16, tag="o")
                    nc.vector.tensor_scalar_add(
                        o_t[:], pss[b][:], ba_t[:, j:j + 1])
                    # alternate eviction queues: halves the per-queue
                    # writeback backlog (and the end-of-kernel drain)
                    eng = nc.scalar if b % 2 else nc.gpsimd
                    eng.dma_start(
                        out=outT[j * 128:(j + 1) * 128,
                                 tt * NT:(tt + 1) * NT],
                        in_=o_t[:],
                    )

            # ---- software-pipelined sweep (depth 2 at the head so the
            # PE chews shrink work while W streams in): shrink(tt+2)
            # hides inside main(tt), so the x8 stream never stalls the
            # PE and main work starts ~20us in ----
            shrink(0)
            shrink(1)
            load_w(0)
            x_t = load_x(0)
            load_w(1)
            load_w(2)
            for tt in range(ntt):
                main_group(tt, 0, x_t)
                main_group(tt, 1, x_t)
                if tt + 2 < ntt:
                    shrink(tt + 2)
                x_next = load_x(tt + 1) if tt + 1 < ntt else None
                main_group(tt, 2, x_t)
                x_t = x_next
    nc.compile()
    return nc


_nc_cache = {}


def _get_program(tc_tokens=TC):
    if tc_tokens not in _nc_cache:
        _nc_cache[tc_tokens] = build_program(tc_tokens)
    return _nc_cache[tc_tokens]


def make_in_maps(x, W_qkv, bias_qkv, lora_a_q, lora_a_k, lora_a_v,
                 lora_b_q, lora_b_k, lora_b_v,
                 lora_bias_q, lora_bias_k, lora_bias_v,
                 token_lora_indices, ncores=NCORES):
    x = np.asarray(x, np.float32)
    idx = np.asarray(token_lora_indices).astype(np.int64)
    tc_tokens = x.shape[0] // ncores

    # W resident: [128, NKT, O], p-major over the contraction dim
    wres = np.ascontiguousarray(
        np.asarray(W_qkv, np.float32).T.reshape(NKT, 128, O)
        .transpose(1, 0, 2)).astype(BF16NP)

    # lora A stacked (q|k|v) -> [128, NQP, 2, 384], scaled x8 for fp8
    a_stack = np.concatenate([
        np.asarray(lora_a_q, np.float32).reshape(L * R, D),
        np.asarray(lora_a_k, np.float32).reshape(L * R, D),
        np.asarray(lora_a_v, np.float32).reshape(L * R, D)], axis=0)
    a8 = np.ascontiguousarray(
        (a_stack.T * 8.0).reshape(NQP, 2, 128, 384)
        .transpose(2, 0, 1, 3)).astype(FP8NP)

    # fused expand weights: plane 0 = 8*bcomb, plane 1 = 8*biasL
    # zero-padded to 128 partitions
    bcomb = np.concatenate([
        np.asarray(lora_b_q, np.float32).transpose(0, 2, 1).reshape(L * R, QS),
        np.asarray(lora_b_k, np.float32).transpose(0, 2, 1).reshape(L * R, KVS),
        np.asarray(lora_b_v, np.float32).transpose(0, 2, 1).reshape(L * R, KVS)],
        axis=1)                                     # (128, O)
    biasL = np.concatenate([
        np.asarray(lora_bias_q, np.float32),
        np.asarray(lora_bias_k, np.float32),
        np.asarray(lora_bias_v, np.float32)], axis=1)  # (L, O)
    biasLpad = np.zeros((128, O), np.float32)
    biasLpad[:L] = biasL
    wc8 = np.empty((128, NBLK, 2, 128), np.float32)
    wc8[:, :, 0, :] = (8.0 * bcomb).reshape(128, NBLK, 128)
    wc8[:, :, 1, :] = (8.0 * biasLpad).reshape(128, NBLK, 128)
    wc8 = wc8.astype(FP8NP)

    bias_arr = np.ascontiguousarray(
        np.asarray(bias_qkv, np.float32).reshape(NBLK, 128).T)
    lane = np.arange(128) // R

    xT_all = np.asarray(x, np.float32).T            # (D, T)
    ntt = tc_tokens // NT
    in_maps = []
    for c in range(ncores):
        sl = slice(c * tc_tokens, (c + 1) * tc_tokens)
        idx_c = idx[sl]
        xT = xT_all[:, sl]                          # (D, tc)
        ohpad = np.zeros((128, tc_tokens), np.float32)
        ohpad[:L] = (idx_c[None, :] == np.arange(L)[:, None]) / 8.0
        # xb[tt, p, k, n] = x^T[k*128+p, tt*NT+n]
        xb = np.ascontiguousarray(
            xT.reshape(NKT, 128, ntt, NT).transpose(2, 1, 0, 3)
        ).astype(BF16NP)
        # x8m[tt, p, 2q+i, n] = x^T[q*256+i*128+p, tt*NT+n]; plane 16 = mask
        x8m = np.empty((ntt, 128, NQP * 2 + 1, NT), FP8NP)
        x8m[:, :, :NQP * 2, :] = (
            xT.reshape(NQP * 2, 128, ntt, NT).transpose(2, 1, 0, 3)
        ).astype(FP8NP)
        x8m[:, :, NQP * 2, :] = (
            ((idx_c[None, :] == lane[:, None]) / 64.0).astype(FP8NP)
            .reshape(128, ntt, NT).transpose(1, 0, 2))
        in_maps.append({
            "xb": xb,
            "x8m": x8m,
            "wres": wres,
            "a8": a8,
            "wc8": wc8,
            "oh8": ohpad.astype(FP8NP),
            "bias_arr": bias_arr,
        })
    return in_maps, tc_tokens


def kernel(x, W_qkv, bias_qkv, lora_a_q, lora_a_k, lora_a_v,
           lora_b_q, lora_b_k, lora_b_v,
           lora_bias_q, lora_bias_k, lora_bias_v,
           token_lora_indices):
    in_maps, tc_tokens = make_in_maps(
        x, W_qkv, bias_qkv, lora_a_q, lora_a_k, lora_a_v,
        lora_b_q, lora_b_k, lora_b_v,
        lora_bias_q, lora_bias_k, lora_bias_v, token_lora_indices)
    nc = _get_program(tc_tokens)
    res = run_bass_kernel_spmd(nc, in_maps, list(range(NCORES)))
    out = np.empty((T, O), np.float32)
    for c in range(NCORES):
        out[c * tc_tokens:(c + 1) * tc_tokens] = \
            res.results[c]["outT"].T.astype(np.float32)
    return out
